# revision 7
# baseline (speedup 1.0000x reference)
"""GCN (2-layer) + mean-pool + MLP head on 8 TRN2 NeuronCores.

Strategy (dst-sharded graph partitioning):
- Nodes sharded 8 ways; core c owns nodes [c*NLOC, (c+1)*NLOC) and all edges
  whose dst lands in its shard (plus per-node self-loops).
- GCN normalization factorizes: out[v] = dis[v]*(sum_e dis[src]*h[src] +
  dis[v]*h[v]) + b with dis = 1/sqrt(deg+1), so messages are gathered from a
  raw (unscaled) feature table and dis[src] rides in the aggregation one-hot.
- Layer tables: L1 gathers from x directly (replicated to every core, bf16);
  L2 gathers from AllGather(relu(layer1 out)).
- Edge aggregation: edges sorted by (src-group, dst-chunk); dma_gather pulls
  128-row tiles of messages; a per-tile weighted one-hot ([128e x 128dst],
  value dis[src]) is built on DVE with iota/is_equal and contracted on the
  TensorEngine into a per-chunk PSUM accumulator, drained into an SBUF f32
  accumulator (4 source groups because dma_gather indices are int16).
- Per-chunk transform: scale by dis[dst], PE-transpose, matmul with W (+bias
  via a rank-1 ones matmul), relu.
- Mean-pool via batch one-hot matmuls into per-core partials + AllReduce;
  MLP head computed redundantly on every core; core 0's output returned.
"""
import sys
sys.path.insert(0, '/opt/trn_rl_repo')
import contextlib
import numpy as np
import ml_dtypes

import concourse.bass as bass
import concourse.bacc as bacc
import concourse.mybir as mybir
import concourse.tile as tile
from concourse import library_config
from concourse.bass_utils import run_bass_kernel_spmd

BF16 = ml_dtypes.bfloat16
CORES = 8
F = 128          # feature/hidden width (fixed at 128 = partition width)
NGRP = 4         # src groups (int16 gather index limit)
CALL_TILES = 100  # tiles (of 128 rows) per dma_gather call


class Geom:
    def __init__(self, n_nodes=100000, n_edges=1600000, n_graphs=64, a_dim=8):
        assert n_nodes % (CORES * NGRP) == 0 or True
        self.N = n_nodes
        self.E = n_edges
        self.G = n_graphs
        self.A = a_dim
        self.NLOC = n_nodes // CORES
        self.GRP = n_nodes // NGRP
        assert self.GRP <= 32767, "int16 gather index limit"
        self.CH = (self.NLOC + 127) // 128  # dst chunks per core


def _prep(geom, x, edge_index, batch, W1, b1, W2, b2, fc1_w, fc1_b, fc2_w, fc2_b):
    """Host-side preprocessing: degrees, edge sharding/sorting, padding plan,
    per-core input arrays."""
    g_ = geom
    N, NLOC, GRP, CH = g_.N, g_.NLOC, g_.GRP, g_.CH
    src = np.asarray(edge_index[0], dtype=np.int64)
    dst = np.asarray(edge_index[1], dtype=np.int64)
    batch = np.asarray(batch, dtype=np.int64)

    deg = np.bincount(dst, minlength=N).astype(np.float32) + 1.0
    dis = (1.0 / np.sqrt(deg)).astype(np.float32)

    core_of = dst // NLOC
    per_core = []
    cnt = np.zeros((CORES, NGRP * CH), np.int64)
    vloc = np.arange(NLOC, dtype=np.int64)
    for c in range(CORES):
        m = core_of == c
        s = np.concatenate([src[m], vloc + c * NLOC])
        d = np.concatenate([dst[m] - c * NLOC, vloc])
        w = dis[s].astype(np.float32)  # self weight = dis[v] as well
        sg = s // GRP
        ch = d >> 7
        sl = (d & 127).astype(np.int64)
        seg = sg * CH + ch
        order = np.argsort(seg, kind='stable')
        per_core.append((s[order], seg[order], sl[order], w[order]))
        cnt[c] = np.bincount(seg, minlength=NGRP * CH)

    L = cnt.max(axis=0)
    L = np.maximum(((L + 127) // 128) * 128, 128)  # [NGRP*CH]
    base = np.zeros(NGRP * CH + 1, np.int64)
    np.cumsum(L, out=base[1:])
    S_total = int(base[-1])
    TT = S_total // 128

    # tile metadata (identical across cores)
    chunk_of_tile = np.zeros(TT, np.int32)
    first_of_tile = np.zeros(TT, bool)
    last_of_tile = np.zeros(TT, bool)
    grp_starts = []  # tile index where each group's stream starts
    for gidx in range(NGRP):
        grp_starts.append(int(base[gidx * CH] // 128))
        for ch in range(CH):
            segi = gidx * CH + ch
            t0, t1 = base[segi] // 128, base[segi + 1] // 128
            chunk_of_tile[t0:t1] = ch
            first_of_tile[t0] = True
            last_of_tile[t1 - 1] = True
    grp_tiles = [int(L[gidx * CH:(gidx + 1) * CH].sum() // 128) for gidx in range(NGRP)]

    # call plan per group: list of tile counts
    call_plan = []
    for gidx in range(NGRP):
        left = grp_tiles[gidx]
        calls = []
        while left > 0:
            take = min(CALL_TILES, left)
            calls.append(take)
            left -= take
        call_plan.append(calls)

    # per-core streams
    in_maps = []
    counts = np.bincount(batch, minlength=g_.G).astype(np.float32)
    invc = (1.0 / np.maximum(counts, 1.0)).astype(np.float32).reshape(g_.G, 1)
    xt = np.asarray(x, dtype=np.float32).astype(BF16)
    pad_nodes = CH * 128 - NLOC
    for c in range(CORES):
        s, seg, sl, w = per_core[c]
        # destination positions in padded stream
        seg_start_in_sorted = np.searchsorted(seg, np.arange(NGRP * CH))
        rank = np.arange(len(seg)) - seg_start_in_sorted[seg]
        pos = base[seg] + rank
        idxv = np.zeros(S_total, np.int16)
        dstlv = np.full(S_total, -1.0, np.float32)
        wgtv = np.zeros(S_total, np.float32)
        idxv[pos] = (s - (s // GRP) * GRP).astype(np.int16)
        dstlv[pos] = sl
        wgtv[pos] = w
        im = {}
        for gidx in range(NGRP):
            lo, hi = int(base[gidx * CH]), int(base[(gidx + 1) * CH])
            seg16 = idxv[lo:hi].reshape(-1, 16).T  # [16, n/16]
            im[f"idxg{gidx}"] = np.tile(seg16, (8, 1)).copy()
        im["dstl"] = dstlv.reshape(TT, 128).T.copy()
        im["wgt"] = wgtv.reshape(TT, 128).T.copy()
        disl = np.zeros(CH * 128, np.float32)
        disl[:NLOC] = dis[c * NLOC:(c + 1) * NLOC]
        im["dist"] = disl.reshape(CH, 128).T.copy()
        bl = np.full(CH * 128, -1.0, np.float32)
        bl[:NLOC] = batch[c * NLOC:(c + 1) * NLOC]
        im["batchv"] = bl.reshape(CH, 128).T.copy()
        im["invc"] = invc
        im["xt"] = xt
        im["iota"] = np.tile(np.arange(128, dtype=np.float32), (128, 1)).astype(BF16)
        im["iotab"] = np.tile(np.arange(g_.G, dtype=np.float32), (128, 1)).astype(BF16)
        im["ident"] = np.eye(128, dtype=np.float32).astype(BF16)
        im["w1"] = np.asarray(W1, np.float32).astype(BF16)
        im["w2"] = np.asarray(W2, np.float32).astype(BF16)
        im["fc1w"] = np.asarray(fc1_w, np.float32).astype(BF16)
        im["fc2w"] = np.asarray(fc2_w, np.float32).astype(BF16)
        im["b1"] = np.asarray(b1, np.float32).astype(BF16).reshape(1, F)
        im["b2"] = np.asarray(b2, np.float32).astype(BF16).reshape(1, F)
        im["fc2b"] = np.asarray(fc2_b, np.float32).astype(BF16).reshape(1, g_.A)
        im["fc1b"] = np.asarray(fc1_b, np.float32).reshape(F, 1).copy()
        in_maps.append(im)

    plan = dict(
        TT=TT, grp_tiles=grp_tiles, call_plan=call_plan,
        chunk_of_tile=chunk_of_tile, first_of_tile=first_of_tile,
        last_of_tile=last_of_tile,
    )
    return plan, in_maps


def _build(geom, plan, tag="", stages="all"):
    g_ = geom
    N, NLOC, GRP, CH, G, A = g_.N, g_.NLOC, g_.GRP, g_.CH, g_.G, g_.A
    TT = plan["TT"]
    bf = mybir.dt.bfloat16
    f32 = mybir.dt.float32
    AL = mybir.AluOpType
    ACT = mybir.ActivationFunctionType

    nc = bacc.Bacc("TRN2", debug=False, target_bir_lowering=False)
    P = {}
    def par(name, shape, dt):
        P[name] = nc.declare_dram_parameter(name + tag, list(shape), dt, isOutput=False)
        return P[name]

    for gidx in range(NGRP):
        par(f"idxg{gidx}", [128, plan["grp_tiles"][gidx] * 8], mybir.dt.int16)
    par("dstl", [128, TT], f32)
    par("wgt", [128, TT], f32)
    par("dist", [128, CH], f32)
    par("batchv", [128, CH], f32)
    par("invc", [G, 1], f32)
    par("xt", [N, F], bf)
    par("iota", [128, 128], bf)
    par("iotab", [128, G], bf)
    par("ident", [128, 128], bf)
    par("w1", [F, F], bf)
    par("w2", [F, F], bf)
    par("fc1w", [F, F], bf)
    par("fc2w", [F, A], bf)
    par("b1", [1, F], bf)
    par("b2", [1, F], bf)
    par("fc2b", [1, A], bf)
    par("fc1b", [F, 1], f32)
    out_ext = nc.declare_dram_parameter("out" + tag, [G, A], f32, isOutput=True)

    agin = nc.dram_tensor("agin" + tag, [NLOC, F], bf)
    tbl2 = nc.dram_tensor("tbl2" + tag, [N, F], bf, addr_space="Shared")
    ar_in = nc.dram_tensor("arin" + tag, [G, F], f32)
    ar_out = nc.dram_tensor("arout" + tag, [G, F], f32, addr_space="Shared")

    with tile.TileContext(nc) as tc:
        with contextlib.ExitStack() as ex:
            pc = ex.enter_context(tc.tile_pool(name="const", bufs=1))
            pacc_pool = ex.enter_context(tc.tile_pool(name="accp", bufs=1))
            pidx = ex.enter_context(tc.tile_pool(name="idx", bufs=2))
            pg = ex.enter_context(tc.tile_pool(name="gbuf", bufs=2))
            poh = ex.enter_context(tc.tile_pool(name="oh", bufs=4))
            ptf = ex.enter_context(tc.tile_pool(name="tf", bufs=3))
            pseg = ex.enter_context(tc.tile_pool(name="pseg", bufs=2, space=bass.MemorySpace.PSUM))
            ptp = ex.enter_context(tc.tile_pool(name="ptp", bufs=6, space=bass.MemorySpace.PSUM))

            nc.gpsimd.load_library(library_config.mlp)

            # load constants
            ct = {}
            for nm in ["dstl", "wgt", "dist", "batchv", "iota", "iotab", "ident",
                       "w1", "w2", "fc1w", "fc2w", "b1", "b2", "fc2b", "fc1b", "invc"]:
                t = pc.tile([P[nm].shape[0], P[nm].shape[1]], P[nm].dtype, tag=nm)
                nc.sync.dma_start(out=t[:], in_=P[nm][:, :])
                ct[nm] = t
            ones = pc.tile([1, 128], bf)
            nc.vector.memset(ones[:], 1.0)

            acc = pacc_pool.tile([128, CH * 128], f32)
            pacc = pacc_pool.tile([G, F], f32)

            chunk_of = plan["chunk_of_tile"]
            first_of = plan["first_of_tile"]
            last_of = plan["last_of_tile"]

            layers = (1,) if stages in ("edge1", "l1") else (1, 2)
            for layer in layers:
                tblap = P["xt"].ap() if layer == 1 else tbl2.ap()
                wt = ct["w1"] if layer == 1 else ct["w2"]
                bt = ct["b1"] if layer == 1 else ct["b2"]

                # -------- edge phase --------
                t_global = 0
                ps = None
                for gidx in range(NGRP):
                    pos16 = 0  # column offset into idxg (16-wrapped)
                    for ntiles in plan["call_plan"][gidx]:
                        nidx = ntiles * 128
                        idx_t = pidx.tile([128, nidx // 16], mybir.dt.int16)
                        nc.sync.dma_start(
                            out=idx_t[:],
                            in_=P[f"idxg{gidx}"][:, pos16:pos16 + nidx // 16])
                        gbuf = pg.tile([128, ntiles, F], bf)
                        nc.gpsimd.dma_gather(
                            gbuf[:], tblap[gidx * GRP:(gidx + 1) * GRP],
                            idx_t[:], nidx, nidx, F, single_packet=False)
                        for tt in range(ntiles):
                            t = t_global
                            ch = int(chunk_of[t])
                            oh = poh.tile([128, 128], bf)
                            nc.vector.tensor_scalar(
                                oh[:], ct["iota"][:],
                                ct["dstl"][:, t:t + 1], ct["wgt"][:, t:t + 1],
                                AL.is_equal, AL.mult)
                            if first_of[t]:
                                ps = pseg.tile([128, 128], f32)
                            nc.tensor.matmul(ps[:], oh[:], gbuf[:, tt, :],
                                             start=bool(first_of[t]),
                                             stop=bool(last_of[t]))
                            if last_of[t]:
                                csl = acc[:, ch * 128:(ch + 1) * 128]
                                if gidx == 0:
                                    nc.vector.tensor_copy(csl, ps[:])
                                else:
                                    nc.vector.tensor_tensor(csl, csl, ps[:], AL.add)
                            t_global += 1
                        pos16 += nidx // 16

                # -------- transform phase --------
                for ch in (range(CH) if stages != "edge1" else range(0)):
                    rows = min(128, NLOC - ch * 128)
                    aggS = ptf.tile([128, 128], bf)
                    nc.vector.tensor_scalar(
                        aggS[:], acc[:, ch * 128:(ch + 1) * 128],
                        ct["dist"][:, ch:ch + 1], None, AL.mult)
                    psT = ptp.tile([128, 128], bf, tag="ps")
                    nc.tensor.transpose(psT[:], aggS[:], ct["ident"][:])
                    aggT = ptf.tile([128, 128], bf)
                    nc.scalar.copy(aggT[:], psT[:])
                    psO = ptp.tile([128, 128], f32, tag="ps")
                    nc.tensor.matmul(psO[:], aggT[:], wt[:], start=True, stop=False)
                    nc.tensor.matmul(psO[:], ones[:1, :], bt[:1, :], start=False, stop=True)
                    rel = ptf.tile([128, 128], bf)
                    nc.scalar.activation(rel[:], psO[:], ACT.Relu)
                    if layer == 1:
                        nc.sync.dma_start(out=agin[ch * 128:ch * 128 + rows, :],
                                          in_=rel[:rows, :])
                    else:
                        ohb = ptf.tile([128, G], bf)
                        nc.vector.tensor_scalar(
                            ohb[:], ct["iotab"][:],
                            ct["batchv"][:, ch:ch + 1], None, AL.is_equal)
                        psB = ptp.tile([G, F], f32, tag="ps")
                        nc.tensor.matmul(psB[:], ohb[:], rel[:], start=True, stop=True)
                        if ch == 0:
                            nc.vector.tensor_copy(pacc[:], psB[:])
                        else:
                            nc.vector.tensor_tensor(pacc[:], pacc[:], psB[:], AL.add)

                if layer == 1 and stages not in ("edge1", "l1noag"):
                    nc.gpsimd.collective_compute(
                        "AllGather", AL.bypass,
                        ins=[agin.ap().opt()], outs=[tbl2.ap().opt()],
                        replica_groups=[list(range(CORES))])

            # -------- pooling + head --------
            if stages in ("edge1", "l1", "l1noag"):
                z0 = ptf.tile([G, A], f32)
                nc.vector.memset(z0[:], 0.0)
                nc.sync.dma_start(out=out_ext[:, :], in_=z0[:])
            else:
                nc.sync.dma_start(out=ar_in[:, :], in_=pacc[:])
                nc.gpsimd.collective_compute(
                    "AllReduce", AL.add,
                    ins=[ar_in.ap().opt()], outs=[ar_out.ap().opt()],
                    replica_groups=[list(range(CORES))])
                pooledf = ptf.tile([G, F], f32)
                nc.sync.dma_start(out=pooledf[:], in_=ar_out[:, :])
                pooledb = ptf.tile([G, F], bf)
                nc.vector.tensor_scalar(pooledb[:], pooledf[:], ct["invc"][:, :1],
                                        None, AL.mult)
                psPT = ptp.tile([F, G], bf, tag="ps")
                nc.tensor.transpose(psPT[:], pooledb[:], ct["ident"][:G, :G])
                pooledT = ptf.tile([F, G], bf)
                nc.scalar.copy(pooledT[:], psPT[:])
                psZ = ptp.tile([F, G], f32, tag="ps")
                nc.tensor.matmul(psZ[:], ct["fc1w"][:], pooledT[:], start=True, stop=True)
                zT = ptf.tile([F, G], bf)
                nc.scalar.activation(zT[:], psZ[:], ACT.Relu, bias=ct["fc1b"][:, :1])
                psO2 = ptp.tile([G, A], f32, tag="ps")
                nc.tensor.matmul(psO2[:], zT[:], ct["fc2w"][:], start=True, stop=False)
                nc.tensor.matmul(psO2[:], ones[:1, :G], ct["fc2b"][:1, :],
                                 start=False, stop=True)
                outt = ptf.tile([G, A], f32)
                nc.scalar.activation(outt[:], psO2[:], ACT.Sigmoid)
                nc.sync.dma_start(out=out_ext[:, :], in_=outt[:])

    nc.compile()
    return nc


_GEOM = Geom()


def kernel(x, edge_index, batch, W1, b1, W2, b2, fc1_w, fc1_b, fc2_w, fc2_b):
    plan, in_maps = _prep(_GEOM, x, edge_index, batch, W1, b1, W2, b2,
                          fc1_w, fc1_b, fc2_w, fc2_b)
    nc = _build(_GEOM, plan)
    res = run_bass_kernel_spmd(nc, in_maps, list(range(CORES)))
    return np.asarray(res.results[0]["out"], dtype=np.float32)


# revision 11
# speedup vs baseline: 4.7339x; 4.7339x over previous
"""GCN (2-layer) + mean-pool + MLP head on 8 TRN2 NeuronCores.

Strategy (dst-sharded graph partitioning):
- Nodes sharded 8 ways; core c owns nodes [c*NLOC, (c+1)*NLOC) and all edges
  whose dst lands in its shard (plus per-node self-loops).
- GCN normalization factorizes: out[v] = dis[v]*(sum_e dis[src]*h[src] +
  dis[v]*h[v]) + b with dis = 1/sqrt(deg+1), so messages are gathered from a
  raw (unscaled) feature table and dis[src] rides in the aggregation one-hot.
- Layer tables: L1 gathers from x directly (replicated to every core, bf16);
  L2 gathers from AllGather(relu(layer1 out)).
- Edge aggregation: edges sorted by (src-group, dst-chunk); dma_gather pulls
  128-row tiles of messages; a per-tile weighted one-hot ([128e x 128dst],
  value dis[src]) is built on DVE with iota/is_equal and contracted on the
  TensorEngine into a per-chunk PSUM accumulator, drained into an SBUF f32
  accumulator (4 source groups because dma_gather indices are int16).
- Per-chunk transform: scale by dis[dst], PE-transpose, matmul with W (+bias
  via a rank-1 ones matmul), relu.
- Mean-pool via batch one-hot matmuls into per-core partials + AllReduce;
  MLP head computed redundantly on every core; core 0's output returned.
"""
import sys
sys.path.insert(0, '/opt/trn_rl_repo')
import contextlib
import numpy as np
import ml_dtypes

import concourse.bass as bass
import concourse.bacc as bacc
import concourse.mybir as mybir
import concourse.tile as tile
from concourse import library_config
from concourse.bass_utils import run_bass_kernel_spmd

BF16 = ml_dtypes.bfloat16
CORES = 8
F = 128          # feature/hidden width (fixed at 128 = partition width)
NGRP = 4         # src groups (int16 gather index limit)
CALL_TILES = 64   # tiles (of 128 rows) per dma_gather call


class Geom:
    def __init__(self, n_nodes=100000, n_edges=1600000, n_graphs=64, a_dim=8):
        assert n_nodes % (CORES * NGRP) == 0 or True
        self.N = n_nodes
        self.E = n_edges
        self.G = n_graphs
        self.A = a_dim
        self.NLOC = n_nodes // CORES
        self.GRP = n_nodes // NGRP
        assert self.GRP <= 32767, "int16 gather index limit"
        self.CH = (self.NLOC + 127) // 128  # dst chunks per core


def _prep(geom, x, edge_index, batch, W1, b1, W2, b2, fc1_w, fc1_b, fc2_w, fc2_b):
    """Host-side preprocessing: degrees, edge sharding/sorting, padding plan,
    per-core input arrays."""
    g_ = geom
    N, NLOC, GRP, CH = g_.N, g_.NLOC, g_.GRP, g_.CH
    src = np.asarray(edge_index[0], dtype=np.int64)
    dst = np.asarray(edge_index[1], dtype=np.int64)
    batch = np.asarray(batch, dtype=np.int64)

    deg = np.bincount(dst, minlength=N).astype(np.float32) + 1.0
    dis = (1.0 / np.sqrt(deg)).astype(np.float32)

    assert NLOC % NGRP == 0
    BND = NLOC // NGRP
    # band-major table layout: node u=(r,i) -> row (i//BND)*GRP + r*BND + i%BND
    # so AllGather stage b fills exactly table rows [b*GRP,(b+1)*GRP) = group b
    u = np.arange(N, dtype=np.int64)
    r_, i_ = u // NLOC, u % NLOC
    row_of_node = (i_ // BND) * GRP + r_ * BND + (i_ % BND)
    node_of_row = np.empty(N, np.int64)
    node_of_row[row_of_node] = u

    core_of = dst // NLOC
    per_core = []
    cnt = np.zeros((CORES, NGRP * CH), np.int64)
    vloc = np.arange(NLOC, dtype=np.int64)
    for c in range(CORES):
        m = core_of == c
        s = np.concatenate([src[m], vloc + c * NLOC])
        d = np.concatenate([dst[m] - c * NLOC, vloc])
        w = (dis[s] * dis[np.concatenate([dst[m], vloc + c * NLOC])]).astype(np.float32)  # dis_src*dis_dst (self: dis_v^2)
        s = row_of_node[s]  # table rows, band-major
        sg = s // GRP
        ch = d >> 7
        sl = (d & 127).astype(np.int64)
        seg = sg * CH + ch
        order = np.argsort(seg, kind='stable')
        per_core.append((s[order], seg[order], sl[order], w[order]))
        cnt[c] = np.bincount(seg, minlength=NGRP * CH)

    L = cnt.max(axis=0)
    L = np.maximum(((L + 127) // 128) * 128, 128)  # [NGRP*CH]
    base = np.zeros(NGRP * CH + 1, np.int64)
    np.cumsum(L, out=base[1:])
    S_total = int(base[-1])
    TT = S_total // 128

    # tile metadata (identical across cores)
    chunk_of_tile = np.zeros(TT, np.int32)
    first_of_tile = np.zeros(TT, bool)
    last_of_tile = np.zeros(TT, bool)
    grp_starts = []  # tile index where each group's stream starts
    for gidx in range(NGRP):
        grp_starts.append(int(base[gidx * CH] // 128))
        for ch in range(CH):
            segi = gidx * CH + ch
            t0, t1 = base[segi] // 128, base[segi + 1] // 128
            chunk_of_tile[t0:t1] = ch
            first_of_tile[t0] = True
            last_of_tile[t1 - 1] = True
    grp_tiles = [int(L[gidx * CH:(gidx + 1) * CH].sum() // 128) for gidx in range(NGRP)]

    # call plan per group: list of tile counts
    call_plan = []
    for gidx in range(NGRP):
        left = grp_tiles[gidx]
        calls = []
        while left > 0:
            take = min(CALL_TILES, left)
            calls.append(take)
            left -= take
        call_plan.append(calls)

    # per-core streams
    in_maps = []
    counts = np.bincount(batch, minlength=g_.G).astype(np.float32)
    invc = (1.0 / np.maximum(counts, 1.0)).astype(np.float32).reshape(g_.G, 1)
    xt = np.asarray(x, dtype=np.float32).astype(BF16)[node_of_row]  # band-major rows
    pad_nodes = CH * 128 - NLOC
    for c in range(CORES):
        s, seg, sl, w = per_core[c]
        # destination positions in padded stream
        seg_start_in_sorted = np.searchsorted(seg, np.arange(NGRP * CH))
        rank = np.arange(len(seg)) - seg_start_in_sorted[seg]
        pos = base[seg] + rank
        idxv = np.zeros(S_total, np.int16)
        idxv[pos] = (s - (s // GRP) * GRP).astype(np.int16)
        im = {}
        for gidx in range(NGRP):
            lo, hi = int(base[gidx * CH]), int(base[(gidx + 1) * CH])
            seg16 = idxv[lo:hi].reshape(-1, 16).T  # [16, n/16]
            im[f"idxg{gidx}"] = np.tile(seg16, (8, 1)).copy()
        # precomputed weighted one-hot tiles: A[t, e, d] = norm weight
        A = np.zeros(S_total * 128, BF16)
        A[pos * 128 + sl] = w.astype(BF16)
        im["oh"] = np.ascontiguousarray(
            A.reshape(TT, 128, 128).transpose(1, 0, 2).reshape(128, TT * 128))
        # precomputed batch one-hots: B[ch, node_slot, graph]
        B = np.zeros(CH * 128 * g_.G, BF16)
        bl = batch[c * NLOC:(c + 1) * NLOC]
        nodepos = np.arange(NLOC)
        B[nodepos * g_.G + bl] = np.float32(1.0)
        im["ohb"] = np.ascontiguousarray(
            B.reshape(CH, 128, g_.G).transpose(1, 0, 2).reshape(128, CH * g_.G))
        im["invc"] = invc
        im["xt"] = xt
        im["ident"] = np.eye(128, dtype=np.float32).astype(BF16)
        im["w1"] = np.asarray(W1, np.float32).astype(BF16)
        im["w2"] = np.asarray(W2, np.float32).astype(BF16)
        im["fc1w"] = np.asarray(fc1_w, np.float32).astype(BF16)
        im["fc2w"] = np.asarray(fc2_w, np.float32).astype(BF16)
        im["b1"] = np.asarray(b1, np.float32).astype(BF16).reshape(1, F)
        im["b2"] = np.asarray(b2, np.float32).astype(BF16).reshape(1, F)
        im["fc2b"] = np.asarray(fc2_b, np.float32).astype(BF16).reshape(1, g_.A)
        im["fc1b"] = np.asarray(fc1_b, np.float32).reshape(F, 1).copy()
        in_maps.append(im)

    plan = dict(
        TT=TT, grp_tiles=grp_tiles, call_plan=call_plan,
        chunk_of_tile=chunk_of_tile, first_of_tile=first_of_tile,
        last_of_tile=last_of_tile,
    )
    return plan, in_maps


def _build(geom, plan, tag="", stages="all"):
    g_ = geom
    N, NLOC, GRP, CH, G, A = g_.N, g_.NLOC, g_.GRP, g_.CH, g_.G, g_.A
    TT = plan["TT"]
    bf = mybir.dt.bfloat16
    f32 = mybir.dt.float32
    AL = mybir.AluOpType
    ACT = mybir.ActivationFunctionType

    nc = bacc.Bacc("TRN2", debug=False, target_bir_lowering=False)
    P = {}
    def par(name, shape, dt):
        P[name] = nc.declare_dram_parameter(name + tag, list(shape), dt, isOutput=False)
        return P[name]

    for gidx in range(NGRP):
        par(f"idxg{gidx}", [128, plan["grp_tiles"][gidx] * 8], mybir.dt.int16)
    par("oh", [128, TT * 128], bf)
    par("ohb", [128, CH * G], bf)
    par("invc", [G, 1], f32)
    par("xt", [N, F], bf)
    par("ident", [128, 128], bf)
    par("w1", [F, F], bf)
    par("w2", [F, F], bf)
    par("fc1w", [F, F], bf)
    par("fc2w", [F, A], bf)
    par("b1", [1, F], bf)
    par("b2", [1, F], bf)
    par("fc2b", [1, A], bf)
    par("fc1b", [F, 1], f32)
    out_ext = nc.declare_dram_parameter("out" + tag, [G, A], f32, isOutput=True)

    BND = NLOC // NGRP
    agin = [nc.dram_tensor(f"agin{b}" + tag, [BND, F], bf) for b in range(NGRP)]
    tbl2 = [nc.dram_tensor(f"tbl2{b}" + tag, [GRP, F], bf, addr_space="Shared")
            for b in range(NGRP)]
    ar_in = nc.dram_tensor("arin" + tag, [G, F], f32)
    ar_out = nc.dram_tensor("arout" + tag, [G, F], f32, addr_space="Shared")

    with tile.TileContext(nc) as tc:
        with contextlib.ExitStack() as ex:
            pc = ex.enter_context(tc.tile_pool(name="const", bufs=1))
            pacc_pool = ex.enter_context(tc.tile_pool(name="accp", bufs=1))
            pidx = ex.enter_context(tc.tile_pool(name="idx", bufs=2))
            pg = ex.enter_context(tc.tile_pool(name="gbuf", bufs=2))
            poh = ex.enter_context(tc.tile_pool(name="oh", bufs=4))
            ptf = ex.enter_context(tc.tile_pool(name="tf", bufs=3))
            pseg = ex.enter_context(tc.tile_pool(name="pseg", bufs=2, space=bass.MemorySpace.PSUM))
            ptp = ex.enter_context(tc.tile_pool(name="ptp", bufs=6, space=bass.MemorySpace.PSUM))

            nc.gpsimd.load_library(library_config.mlp)

            # load constants
            ct = {}
            for nm in ["ohb", "ident",
                       "w1", "w2", "fc1w", "fc2w", "b1", "b2", "fc2b", "fc1b", "invc"]:
                t = pc.tile([P[nm].shape[0], P[nm].shape[1]], P[nm].dtype, tag=nm)
                nc.sync.dma_start(out=t[:], in_=P[nm][:, :])
                ct[nm] = t
            ones = pc.tile([1, 128], bf)
            nc.vector.memset(ones[:], 1.0)

            acc = pacc_pool.tile([128, CH * 128], f32)
            pacc = pacc_pool.tile([G, F], f32)

            chunk_of = plan["chunk_of_tile"]
            first_of = plan["first_of_tile"]
            last_of = plan["last_of_tile"]

            layers = () if stages == "noop" else ((1,) if stages in ("edge1", "l1", "l1noag") else (1, 2))
            for layer in layers:
                wt = ct["w1"] if layer == 1 else ct["w2"]
                bt = ct["b1"] if layer == 1 else ct["b2"]

                # -------- edge phase --------
                t_global = 0
                ps = None
                for gidx in range(NGRP):
                    pos16 = 0  # column offset into idxg (16-wrapped)
                    for ntiles in plan["call_plan"][gidx]:
                        nidx = ntiles * 128
                        idx_t = pidx.tile([128, nidx // 16], mybir.dt.int16)
                        nc.sync.dma_start(
                            out=idx_t[:],
                            in_=P[f"idxg{gidx}"][:, pos16:pos16 + nidx // 16])
                        gbuf = pg.tile([128, ntiles, F], bf)
                        srcap = (P["xt"].ap()[gidx * GRP:(gidx + 1) * GRP]
                                 if layer == 1 else tbl2[gidx].ap())
                        nc.gpsimd.dma_gather(
                            gbuf[:], srcap,
                            idx_t[:], nidx, nidx, F, single_packet=False)
                        ohslab = poh.tile([128, ntiles, 128], bf)
                        nc.sync.dma_start(
                            out=ohslab[:],
                            in_=P["oh"].ap().rearrange("p (t d) -> p t d", d=128)[:, t_global:t_global + ntiles, :])
                        for tt in range(ntiles):
                            t = t_global
                            ch = int(chunk_of[t])
                            if first_of[t]:
                                ps = pseg.tile([128, 128], f32)
                            nc.tensor.matmul(ps[:], ohslab[:, tt, :], gbuf[:, tt, :],
                                             start=bool(first_of[t]),
                                             stop=bool(last_of[t]))
                            if last_of[t]:
                                csl = acc[:, ch * 128:(ch + 1) * 128]
                                if gidx == 0:
                                    nc.vector.tensor_copy(csl, ps[:])
                                else:
                                    nc.vector.tensor_tensor(csl, csl, ps[:], AL.add)
                            t_global += 1
                        pos16 += nidx // 16

                # -------- transform phase --------
                ag_next = 0
                for ch in (range(CH) if stages != "edge1" else range(0)):
                    rows = min(128, NLOC - ch * 128)
                    aggS = ptf.tile([128, 128], bf)
                    nc.vector.tensor_copy(aggS[:], acc[:, ch * 128:(ch + 1) * 128])
                    psT = ptp.tile([128, 128], bf, tag="ps")
                    nc.tensor.transpose(psT[:], aggS[:], ct["ident"][:])
                    aggT = ptf.tile([128, 128], bf)
                    nc.scalar.copy(aggT[:], psT[:])
                    psO = ptp.tile([128, 128], f32, tag="ps")
                    nc.tensor.matmul(psO[:], aggT[:], wt[:], start=True, stop=False)
                    nc.tensor.matmul(psO[:], ones[:1, :], bt[:1, :], start=False, stop=True)
                    rel = ptf.tile([128, 128], bf)
                    nc.scalar.activation(rel[:], psO[:], ACT.Relu)
                    if layer == 1:
                        lo = ch * 128
                        hi = lo + rows
                        b0, b1 = lo // BND, (hi - 1) // BND
                        for b in range(b0, b1 + 1):
                            s0, s1 = max(lo, b * BND), min(hi, (b + 1) * BND)
                            nc.sync.dma_start(
                                out=agin[b][s0 - b * BND:s1 - b * BND, :],
                                in_=rel[s0 - lo:s1 - lo, :])
                        if stages not in ("edge1", "l1noag"):
                            while ag_next < NGRP and (ag_next + 1) * BND <= hi:
                                nc.gpsimd.collective_compute(
                                    "AllGather", AL.bypass,
                                    ins=[agin[ag_next].ap().opt()],
                                    outs=[tbl2[ag_next].ap().opt()],
                                    replica_groups=[list(range(CORES))])
                                ag_next += 1
                    else:
                        psB = ptp.tile([G, F], f32, tag="ps")
                        nc.tensor.matmul(psB[:], ct["ohb"][:, ch * G:(ch + 1) * G],
                                         rel[:], start=True, stop=True)
                        if ch == 0:
                            nc.vector.tensor_copy(pacc[:], psB[:])
                        else:
                            nc.vector.tensor_tensor(pacc[:], pacc[:], psB[:], AL.add)

                if layer == 1 and stages not in ("edge1", "l1noag"):
                    while ag_next < NGRP:
                        nc.gpsimd.collective_compute(
                            "AllGather", AL.bypass,
                            ins=[agin[ag_next].ap().opt()],
                            outs=[tbl2[ag_next].ap().opt()],
                            replica_groups=[list(range(CORES))])
                        ag_next += 1

            # -------- pooling + head --------
            if stages in ("edge1", "l1", "l1noag", "noop"):
                z0 = ptf.tile([G, A], f32)
                nc.vector.memset(z0[:], 0.0)
                nc.sync.dma_start(out=out_ext[:, :], in_=z0[:])
            else:
                nc.sync.dma_start(out=ar_in[:, :], in_=pacc[:])
                nc.gpsimd.collective_compute(
                    "AllReduce", AL.add,
                    ins=[ar_in.ap().opt()], outs=[ar_out.ap().opt()],
                    replica_groups=[list(range(CORES))])
                pooledf = ptf.tile([G, F], f32)
                nc.sync.dma_start(out=pooledf[:], in_=ar_out[:, :])
                pooledb = ptf.tile([G, F], bf)
                nc.vector.tensor_scalar(pooledb[:], pooledf[:], ct["invc"][:, :1],
                                        None, AL.mult)
                psPT = ptp.tile([F, G], bf, tag="ps")
                nc.tensor.transpose(psPT[:], pooledb[:], ct["ident"][:G, :G])
                pooledT = ptf.tile([F, G], bf)
                nc.scalar.copy(pooledT[:], psPT[:])
                psZ = ptp.tile([F, G], f32, tag="ps")
                nc.tensor.matmul(psZ[:], ct["fc1w"][:], pooledT[:], start=True, stop=True)
                zT = ptf.tile([F, G], bf)
                nc.scalar.activation(zT[:], psZ[:], ACT.Relu, bias=ct["fc1b"][:, :1])
                psO2 = ptp.tile([G, A], f32, tag="ps")
                nc.tensor.matmul(psO2[:], zT[:], ct["fc2w"][:], start=True, stop=False)
                nc.tensor.matmul(psO2[:], ones[:1, :G], ct["fc2b"][:1, :],
                                 start=False, stop=True)
                outt = ptf.tile([G, A], f32)
                nc.scalar.activation(outt[:], psO2[:], ACT.Sigmoid)
                nc.sync.dma_start(out=out_ext[:, :], in_=outt[:])

    nc.compile()
    return nc


_GEOM = Geom()


def kernel(x, edge_index, batch, W1, b1, W2, b2, fc1_w, fc1_b, fc2_w, fc2_b):
    plan, in_maps = _prep(_GEOM, x, edge_index, batch, W1, b1, W2, b2,
                          fc1_w, fc1_b, fc2_w, fc2_b)
    nc = _build(_GEOM, plan)
    res = run_bass_kernel_spmd(nc, in_maps, list(range(CORES)))
    return np.asarray(res.results[0]["out"], dtype=np.float32)
